# revision 13
# baseline (speedup 1.0000x reference)
"""Distance-loss kernel for Trainium2 (8 NeuronCores, data-parallel over batch).

loss = mean over (b, c != label_b) of sqrt(||x_b - center_c||^2)

Host-side staging: x/labels sharded over batch, centers replicated. Matmul
operands are staged in the fp8(e4m3) DoubleRow layouts the device consumes
(x^T, x^T squared elementwise, -2*centers^T); bf16 center rows feed the
class-norm math; labels are staged replicated across partitions. All
reductions, matmuls and sqrts run on device; accumulations are fp32.

Per-core plan (B_shard = 2048 rows, distmat computed as out[c, b]):
  - psum[c, b] = -2 c_c . x_b + ||x_b||^2 via fp8 DoubleRow PE matmuls
    (K=256 contraction per instruction, ~1.4x bf16 rate). The ||x||^2 term
    is an aug matmul ones^T @ (x^T)^2 - the PE both reduces the squares and
    broadcast-adds them, so there is no norm side-chain.
  - d = sqrt(psum + ||c_c||^2): class norm (fp32, from bf16 centers on DVE)
    as the ScalarE ACT per-partition bias; the same instruction accumulates
    sum_b d into the per-core partials and writes d (bf16) to SBUF.
  - label-entry correction WITHOUT any indirect gather (indirect DMA runs
    at ~25-55ns/row - gathering 2048 center rows costs 50-110us of DMA
    latency, which was the hidden critical path of the previous version):
    one DVE STT per class tile computes (labels_rep == cls_col) * d and
    accumulates - extracting exactly the label entries that the fp8 path
    added, straight from the d tiles already in SBUF.
  - A short PE warm-up burst runs while the input DMAs stream.
  - each core returns [128, 16] fp32 partials (cols 0-7: per-class-tile
    sum_b d; cols 8-15: per-class-tile label-entry sums); the host performs
    the final cross-partition/core reduction in float64, subtracts, and
    divides by B*(C-1).
"""

import sys
from contextlib import ExitStack

import numpy as np

if "/opt/trn_rl_repo" not in sys.path:
    sys.path.insert(0, "/opt/trn_rl_repo")

import ml_dtypes

import concourse.bass as bass
import concourse.mybir as mybir
from concourse.bacc import Bacc

F32 = mybir.dt.float32
BF16 = mybir.dt.bfloat16
FP16 = mybir.dt.float16
FP8 = mybir.dt.float8e4
AF = mybir.ActivationFunctionType
ALU = mybir.AluOpType
DR = mybir.MatmulPerfMode.DoubleRow
BF = ml_dtypes.bfloat16
E4 = ml_dtypes.float8_e4m3

N_CORES = 8
B = 16384
C = 1000
D = 256
BS = B // N_CORES          # 2048 rows per core
T = BS // 128              # 16 b-tiles per core
NC_TILES = 8               # ceil(C / 128) class tiles
WU_REPS = 3                # PE warm-up matmuls (pstate ramp during DMA)


def build_nc() -> bass.Bass:
    from concourse.tile import TileContext

    nc = Bacc()
    # xT2: x^T DoubleRow blocks   xT2[p, j, i, n] = x[j*512+n, i*128+p]
    # xq2: elementwise square of x^T (same layout, fp8)
    # cT2: -2*centers^T DoubleRow cT2[p, m, i, q] = -2*centers[m*128+q, i*128+p]
    # cp : center rows, sbuf rows cp[p, i, d] = centers[i*128+p, d] (fp16)
    # lrx: labels replicated + class ids: lrx[p, 0:BS] = label_b,
    #      lrx[p, BS+m] = m*128+p  (fp16)
    xT2_d = nc.dram_tensor("xT2", [128, 2, 2, 512], FP8, kind="ExternalInput")
    xT2b_d = nc.dram_tensor("xT2b", [128, 2, 2, 512], FP8,
                            kind="ExternalInput")
    xq2_d = nc.dram_tensor("xq2", [128, 2, 2, 512], FP8, kind="ExternalInput")
    xq2b_d = nc.dram_tensor("xq2b", [128, 2, 2, 512], FP8,
                            kind="ExternalInput")
    cT2_d = nc.dram_tensor("cT2", [128, NC_TILES, 2, 128], FP8,
                           kind="ExternalInput")
    cp_d = nc.dram_tensor("cp", [128, NC_TILES, D], FP16, kind="ExternalInput")
    lrx_d = nc.dram_tensor("lrx", [128, BS + NC_TILES], FP16,
                           kind="ExternalInput")
    o_d = nc.dram_tensor("out", [128, 16], F32, kind="ExternalOutput")

    with TileContext(nc) as tc, ExitStack() as ctx:
        const = ctx.enter_context(tc.tile_pool(name="const", bufs=1))
        csqp = ctx.enter_context(tc.tile_pool(name="csqp", bufs=2))
        dpool = ctx.enter_context(tc.tile_pool(name="dpool", bufs=2))
        mmps = ctx.enter_context(tc.tile_pool(name="mmps", bufs=2, space="PSUM"))

        # PE warm-up burst while the input DMAs stream
        wu_w = const.tile([128, 128], BF16)
        nc.vector.memset(wu_w[:], 0.5)
        wu_r = const.tile([128, 512], BF16)
        nc.vector.memset(wu_r[:], 0.25)
        wu_ps = mmps.tile([128, 2048], F32, tag="mm")
        for rep in range(WU_REPS):
            nc.tensor.matmul(wu_ps[:, 0:512], wu_w[:], wu_r[:],
                             start=(rep == 0), stop=(rep == WU_REPS - 1))

        # input DMAs: matmul operands on the sync queue, interleaved so each
        # psum column range's main+aug operands land as early as possible;
        # bias/label operands on the scalar queue
        cT2 = const.tile([128, NC_TILES, 2, 128], FP8)
        nc.sync.dma_start(out=cT2[:], in_=cT2_d[:, :, :, :])
        xT2 = const.tile([128, 4, 2, 512], FP8)
        nc.sync.dma_start(out=xT2[:, 0:2, :, :], in_=xT2_d[:, :, :, :])
        xq2 = const.tile([128, 4, 2, 512], FP8)
        nc.sync.dma_start(out=xq2[:, 0:2, :, :], in_=xq2_d[:, :, :, :])
        nc.sync.dma_start(out=xT2[:, 2:4, :, :], in_=xT2b_d[:, :, :, :])
        nc.sync.dma_start(out=xq2[:, 2:4, :, :], in_=xq2b_d[:, :, :, :])
        cp = const.tile([128, NC_TILES, D], FP16)
        nc.scalar.dma_start(out=cp[:], in_=cp_d[:, :, :])
        lrx = const.tile([128, BS + NC_TILES], FP16)
        nc.scalar.dma_start(out=lrx[:], in_=lrx_d[:, :])

        # aug-matmul stationary: ones (fp8 exact)
        ones2 = const.tile([128, 2, 128], FP8)
        nc.vector.memset(ones2[:], 1.0)

        # per-core partials; memset so pad-class rows stay zero
        outp = const.tile([128, 16], F32)
        nc.vector.memset(outp[:], 0.0)

        # ||c||^2 per class (fp32 accumulated from bf16 rows)
        ccP = const.tile([128, NC_TILES], F32)
        for i in range(NC_TILES):
            csq = csqp.tile([128, D], BF16, tag="csq")
            nc.vector.scalar_tensor_tensor(
                out=csq[:], in0=cp[:, i, :], scalar=0.0, in1=cp[:, i, :],
                op0=ALU.bypass, op1=ALU.mult,
                accum_out=ccP[:, i : i + 1],
            )

        # free the warm-up psum slot (cheap DVE read)
        wu_out = const.tile([1, 1], F32)
        nc.vector.tensor_copy(wu_out[:], wu_ps[0:1, 0:1])

        # main loop: per class tile, 4 DoubleRow mains + 4 DoubleRow augs,
        # ScalarE sqrt+bias+accumulate, then the DVE label-entry extract
        for m in range(NC_TILES):
            cnt = min(128, C - m * 128)
            ps = mmps.tile([128, 2048], F32, tag="mm")
            for j in range(4):
                nc.tensor.matmul(
                    ps[:, j * 512 : (j + 1) * 512],
                    cT2[:, m, :, :], xT2[:, j, :, :],
                    start=True, stop=False, perf_mode=DR,
                )
            for j in range(4):
                nc.tensor.matmul(
                    ps[:, j * 512 : (j + 1) * 512],
                    ones2[:, :, :], xq2[:, j, :, :],
                    start=False, stop=True, perf_mode=DR,
                )
            dt_ = dpool.tile([128, 2048], BF16, tag="d")
            nc.scalar.activation(
                dt_[0:cnt, :], ps[0:cnt, :], AF.Sqrt,
                bias=ccP[0:cnt, m : m + 1], scale=1.0,
                accum_out=outp[0:cnt, m : m + 1],
            )
            ext = dpool.tile([128, 2048], BF16, tag="ext")
            nc.vector.scalar_tensor_tensor(
                out=ext[0:cnt, :], in0=lrx[0:cnt, 0:BS],
                scalar=lrx[0:cnt, BS + m : BS + m + 1], in1=dt_[0:cnt, :],
                op0=ALU.is_equal, op1=ALU.mult,
                accum_out=outp[0:cnt, 8 + m : 9 + m],
            )

        nc.sync.dma_start(out=o_d[:, :], in_=outp[:, :])

    nc.compile()
    return nc


_NC_CACHE = None


def _get_nc():
    global _NC_CACHE
    if _NC_CACHE is None:
        _NC_CACHE = build_nc()
    return _NC_CACHE


def make_in_maps(x, centers, labels):
    x = np.asarray(x, dtype=np.float32)
    centers = np.asarray(centers, dtype=np.float32)
    labels = np.asarray(labels)

    cpad = np.zeros((NC_TILES * 128, D), np.float32)
    cpad[:C] = centers
    # cT2[p, m, i, q] = -2*centers[m*128+q, i*128+p]
    cT2 = np.ascontiguousarray(
        (-2.0 * cpad).reshape(NC_TILES, 128, 2, 128).transpose(3, 0, 2, 1)
    ).astype(E4)
    cp = np.ascontiguousarray(
        cpad.reshape(NC_TILES, 128, D).transpose(1, 0, 2)
    ).astype(np.float16)
    cls = np.arange(NC_TILES * 128, dtype=np.float16).reshape(NC_TILES, 128).T

    x8 = x.astype(E4)
    # quantize squares from fp32 x: e4m3 rounding of squares-of-e4m3 values
    # is biased ~-0.8% (squares land below rounding midpoints); from the
    # continuous fp32 values it is ~unbiased.
    xq = np.square(x).astype(E4)

    in_maps = []
    for i in range(N_CORES):
        sl = slice(i * BS, (i + 1) * BS)
        # xT2[p, j, i2, n] = x[j*512+n, i2*128+p]
        xT2 = np.ascontiguousarray(
            x8[sl].reshape(4, 512, 2, 128).transpose(3, 0, 2, 1)
        )
        xq2 = np.ascontiguousarray(
            xq[sl].reshape(4, 512, 2, 128).transpose(3, 0, 2, 1)
        )
        lrx = np.empty((128, BS + NC_TILES), np.float16)
        lrx[:, :BS] = labels[sl].astype(np.float16)[None, :]
        lrx[:, BS:] = cls
        in_maps.append({"xT2": np.ascontiguousarray(xT2[:, 0:2]),
                        "xT2b": np.ascontiguousarray(xT2[:, 2:4]),
                        "xq2": np.ascontiguousarray(xq2[:, 0:2]),
                        "xq2b": np.ascontiguousarray(xq2[:, 2:4]),
                        "cT2": cT2, "cp": cp, "lrx": lrx})
    return in_maps


def reduce_outputs(results):
    total = np.float64(0.0)
    for r in results:
        o = np.asarray(r["out"], dtype=np.float64)
        total += o[:, :8].sum() - o[:, 8:].sum()
    return np.float32(total / (B * (C - 1)))


def _ensure_ntff_hook_module():
    """Provide antenv.axon_hooks if the image's antenv package lacks it.

    concourse.bass_utils imports it for trace=True under axon; the hook
    itself lives in libaxon_pjrt.so and is wrapped by trn_agent_boot.
    """
    import types

    try:
        import antenv.axon_hooks  # noqa: F401
        return
    except ImportError:
        pass
    mod = types.ModuleType("antenv.axon_hooks")
    state = {"hook": None}

    def set_axon_ntff_profile_hook(hook):
        state["hook"] = hook

    def get_axon_ntff_profile_hook():
        if state["hook"] is None:
            try:
                from trn_agent_boot.trn_boot import _ntff_profile_via_ctypes

                state["hook"] = _ntff_profile_via_ctypes(
                    "/opt/axon/libaxon_pjrt.so"
                )
            except Exception:
                return None
        return state["hook"]

    mod.set_axon_ntff_profile_hook = set_axon_ntff_profile_hook
    mod.get_axon_ntff_profile_hook = get_axon_ntff_profile_hook
    sys.modules["antenv.axon_hooks"] = mod
    try:
        import antenv

        antenv.axon_hooks = mod
    except ImportError:
        pass


def kernel(x, centers, labels, _results_out=None, **run_kwargs):
    _ensure_ntff_hook_module()
    from concourse.bass_utils import run_bass_kernel_spmd

    nc = _get_nc()
    in_maps = make_in_maps(x, centers, labels)
    res = run_bass_kernel_spmd(nc, in_maps, core_ids=list(range(N_CORES)),
                               **run_kwargs)
    if _results_out is not None:
        _results_out.append(res)
    return reduce_outputs(res.results)
